# revision 26
# baseline (speedup 1.0000x reference)
"""DelayRNN Trainium2 kernel.

Algorithm notes (derived from the reference semantics):
  - The [B, D+1, D, I] buffer is a write-once delay line: at encoder step t the
    (one-hot) gate picks delay d*, and x_t * 0.99^(d*+1) is read back into
    `mem` exactly at step t + d* + 1.  We therefore keep only an "emergence
    schedule" E[arrival, b, :I] and accumulate outer products into it.
  - Decoder steps shift twice and read after the first shift, so decoder step k
    reads arrival T + 1 + 2k (zero for k >= 16).  Decoder writes are x_zero ->
    the whole decoder gate path is dead code.
  - gumbel noise is a constant (jax key 42); computed on host.
  - rnn_in only feeds the GRU, so W_e2 @ W_ih is fused host-side (W_comb).
  - elu(x) = relu(x) + exp(min(x, 0)) - 1; the -1 is folded into downstream
    biases (colsum of the following weight matrix).

Layout: feature-major [feature-partitions, batch-free], 8-way batch-parallel
(4 rows/core).  The emergence schedule lives in PSUM and is accumulated
directly by the scatter matmuls (start=False).
"""

import sys

import numpy as np

try:
    import concourse.bass  # noqa: F401
except ImportError:
    for p in ("/opt/trn_rl_repo", "/root/.axon_site/_ro/trn_rl_repo"):
        if p not in sys.path:
            sys.path.insert(0, p)

B, T0, I, H, D, C, OUT0 = 32, 128, 64, 256, 32, 64, 32
NCORE = 8
Bc = B // NCORE
DECAY = 0.99

_PROGRAMS = {}
LAST_RESULTS = None


def _build_program(T, OUT):
    import concourse.bass as bass
    import concourse.bacc as bacc
    import concourse.tile as tile
    from concourse import mybir
    from contextlib import ExitStack

    f32 = mybir.dt.float32
    AX = mybir.AxisListType
    OP = mybir.AluOpType
    AF = mybir.ActivationFunctionType

    EMTOT = (T + D + 1) * Bc          # emergence schedule columns (arrival*Bc+b)
    emA_cols = min(EMTOT, 512)
    emB_cols = EMTOT - emA_cols
    N_NZ = min(OUT, (D - 2) // 2 + 1)  # decoder steps with nonzero mem (16)

    nc = bacc.Bacc()

    dI = {}
    def din(name, shape):
        dI[name] = nc.dram_tensor(name, list(shape), f32, kind="ExternalInput")
        return dI[name]

    d_pre_dg = din("pre_dg", [D, T * Bc])
    d_pre_e = din("pre_e", [128, 2, T, Bc])
    d_gp = din("gp", [Bc, T * D])
    d_maskr = din("maskr", [128, T, 2, Bc])
    d_xbm = din("xbm", [Bc, T * I])
    d_eyedecay = din("eyedecay", [Bc, D, Bc])
    d_wdg1h = din("wdg1h", [128, 2, D])
    d_wdg2 = din("wdg2", [D, D])
    d_we1m = din("we1m", [I, 2, 128])
    d_wcomb = din("wcomb", [128, 2, 3 * H])
    d_whh = din("whh", [128, 2, 3 * H])
    d_wf1 = din("wf1", [128, 2, H])
    d_wf2 = din("wf2", [128, 2, C])
    d_bsig = din("bsig", [128, 8])
    d_bdec = din("bdec", [128, 2])
    d_bhhd = din("bhhd", [128, 6])
    d_bcombp = din("bcombp", [128, 6])
    d_bf1 = din("bf1", [128, 2])
    d_bf2 = din("bf2", [C, 1])
    d_zrow = din("zrow", [1, I])
    d_zcols = din("zcols", [1, 512])
    d_out = nc.dram_tensor("out", [C, OUT * Bc], f32, kind="ExternalOutput")

    with ExitStack() as ctx:
        tc = ctx.enter_context(tile.TileContext(nc))

        emerge_pool = ctx.enter_context(
            tc.tile_pool(name="emerge", bufs=1, space="PSUM"))
        emA = emerge_pool.tile([I, emA_cols], f32, tag="emA")
        if emB_cols:
            emB = emerge_pool.tile([I, emB_cols], f32, tag="emB")
        else:
            emB = None

        consts = ctx.enter_context(tc.tile_pool(name="consts", bufs=1))
        state = ctx.enter_context(tc.tile_pool(name="state", bufs=1))
        work = ctx.enter_context(tc.tile_pool(name="work", bufs=3))

        def load(dram, shape):
            t = consts.tile(list(shape), f32, tag=dram.name + "_c")
            nc.sync.dma_start(t[:], dram[:])
            return t

        pre_dg = load(d_pre_dg, [D, T * Bc])
        pre_e = load(d_pre_e, [128, 2, T, Bc])
        gp = load(d_gp, [Bc, T * D])
        maskr = load(d_maskr, [128, T, 2, Bc])
        xbm = load(d_xbm, [Bc, T * I])
        eyedecay = load(d_eyedecay, [Bc, D, Bc])
        wdg1h = load(d_wdg1h, [128, 2, D])
        wdg2 = load(d_wdg2, [D, D])
        we1m = load(d_we1m, [I, 2, 128])
        wcomb = load(d_wcomb, [128, 2, 3 * H])
        whh = load(d_whh, [128, 2, 3 * H])
        wf1 = load(d_wf1, [128, 2, H])
        wf2 = load(d_wf2, [128, 2, C])
        bsig = load(d_bsig, [128, 8])
        bdec = load(d_bdec, [128, 2])
        bhhd = load(d_bhhd, [128, 6])
        bcombp = load(d_bcombp, [128, 6])
        bf1 = load(d_bf1, [128, 2])
        bf2 = load(d_bf2, [C, 1])
        zrow = load(d_zrow, [1, I])
        zcols = load(d_zcols, [1, 512])

        h = state.tile([128, 2, Bc], f32, tag="h")

        nh_all = state.tile([128, 2, OUT, Bc], f32, tag="nh_all")
        dec_mem = state.tile([I, OUT, Bc], f32, tag="dec_mem")
        gidec = state.tile([128, 6, OUT, Bc], f32, tag="gidec")

        nc.gpsimd.memset(h[:], 0.0)

        # zero the emergence schedule (sets has_written for accumulation)
        nc.tensor.matmul(emA[:, 0:emA_cols], zrow[:], zcols[:, 0:emA_cols],
                         start=True, stop=True, skip_group_check=True)
        if emB is not None:
            nc.tensor.matmul(emB[:, 0:emB_cols], zrow[:], zcols[:, 0:emB_cols],
                             start=True, stop=True, skip_group_check=True)

        def em_pieces(col0, ncols):
            """split [col0, col0+ncols) emerge columns into per-tensor pieces"""
            out = []
            if col0 < emA_cols:
                n1 = min(ncols, emA_cols - col0)
                out.append((emA, col0, 0, n1))
                if n1 < ncols:
                    out.append((emB, 0, n1, ncols - n1))
            else:
                out.append((emB, col0 - emA_cols, 0, ncols))
            return out

        # ---------------- encoder ----------------
        with tc.tile_pool(name="penc", bufs=2, space="PSUM") as penc, \
             tc.tile_pool(name="penc1", bufs=1, space="PSUM") as penc1:
            for t in range(T):
                pgate = penc.tile([D, 4 + D], f32, tag="gate")
                p_l1 = pgate[:, 0:Bc]
                p_l2 = pgate[0:Bc, Bc:Bc + D]
                p_e1 = penc1.tile([128, 2, Bc], f32, tag="e1")
                p_grz = penc.tile([128, 4, Bc], f32, tag="grz")
                p_gn = penc1.tile([128, 4, Bc], f32, tag="gn")

                # zero the bank memory on DVE; all matmuls below use
                # start=False (overwrite-on-stale-has_written, then accumulate)
                nc.vector.memset(p_grz[:], 0.0)
                nc.vector.memset(p_gn[:], 0.0)

                # --- PE: l1h first: the gate chain (elu->l2->argmax->G3)
                # runs on ACT/DVE/Pool underneath gh/gi ---
                for k in range(2):
                    nc.tensor.matmul(p_l1, wdg1h[:, k, :], h[:, k, :],
                                     start=(k == 0), stop=(k == 1))

                # --- PE: gh (depends only on h); 4x col-tiling so the
                # stationary loads stream via 4 XBUSes concurrently ---
                for m in range(4):
                    for k in range(2):
                        for j in range(4):
                            nc.tensor.matmul(
                                p_grz[32 * j:32 * (j + 1), m, :],
                                whh[:, k, m * 128 + 32 * j:m * 128 + 32 * (j + 1)],
                                h[:, k, :], start=False, stop=False,
                                skip_group_check=True, tile_position=(0, 32 * j))
                for cch in range(2):
                    for k in range(2):
                        for j in range(4):
                            nc.tensor.matmul(
                                p_gn[32 * j:32 * (j + 1), 2 + cch, :],
                                whh[:, k, (4 + cch) * 128 + 32 * j:(4 + cch) * 128 + 32 * (j + 1)],
                                h[:, k, :], start=False, stop=False,
                                skip_group_check=True, tile_position=(0, 32 * j))
                # mem for this step (written by scatter of step t-1)
                mem_sb = work.tile([I, Bc], f32, tag="mem")
                nc.vector.tensor_copy(mem_sb[:], emA[:, t * Bc:(t + 1) * Bc])

                # --- PE: e1m (col-tiled) ---
                for m in range(2):
                    for j in range(4):
                        nc.tensor.matmul(
                            p_e1[32 * j:32 * (j + 1), m, :],
                            we1m[:, m, 32 * j:32 * (j + 1)], mem_sb[:],
                            start=True, stop=True, skip_group_check=True,
                            tile_position=(0, 32 * j))

                # --- gate path elementwise ---
                l1s = work.tile([D, Bc], f32, tag="l1s")
                nc.vector.tensor_add(l1s[:], p_l1, pre_dg[:, t * Bc:(t + 1) * Bc])
                m1 = work.tile([D, Bc], f32, tag="m1")
                nc.gpsimd.tensor_scalar_min(m1[:], l1s[:], 0.0)
                e1a = work.tile([D, Bc], f32, tag="e1a")
                nc.scalar.activation(e1a[:], m1[:], AF.Exp)
                r1a = work.tile([D, Bc], f32, tag="r1a")
                nc.scalar.activation(r1a[:], l1s[:], AF.Relu)
                celu = work.tile([D, Bc], f32, tag="celu")
                nc.gpsimd.tensor_add(celu[:], e1a[:], r1a[:])

                # --- PE: l2 (logits, batch-major out) ---
                nc.tensor.matmul(p_l2, celu[:], wdg2[:], start=True, stop=True)

                v = work.tile([Bc, D], f32, tag="v")
                nc.vector.tensor_add(v[:], p_l2, gp[:, t * D:(t + 1) * D])
                mx = work.tile([Bc, 1], f32, tag="mx")
                nc.vector.tensor_reduce(mx[:], v[:], axis=AX.X, op=OP.max)
                oh = work.tile([Bc, D], f32, tag="oh")
                nc.vector.tensor_scalar(oh[:], v[:], mx[:], None, op0=OP.is_equal)
                G3 = work.tile([Bc, D, Bc], f32, tag="G3")
                nc.vector.tensor_mul(G3[:], oh[:].broadcast_to((Bc, D, Bc)),
                                     eyedecay[:])

                # --- e path elementwise ---
                e1s = work.tile([128, 2, Bc], f32, tag="e1s")
                nc.vector.tensor_add(e1s[:], p_e1[:], pre_e[:, :, t, :])
                m2 = work.tile([128, 2, Bc], f32, tag="m2")
                nc.gpsimd.tensor_scalar_min(m2[:], e1s[:], 0.0)
                e2a = work.tile([128, 2, Bc], f32, tag="e2a")
                nc.scalar.activation(e2a[:], m2[:], AF.Exp)
                r2a = work.tile([128, 2, Bc], f32, tag="r2a")
                nc.scalar.activation(r2a[:], e1s[:], AF.Relu)
                uprime = work.tile([128, 2, Bc], f32, tag="uprime")
                nc.gpsimd.tensor_add(uprime[:], e2a[:], r2a[:])

                # --- PE: gi ---
                # --- PE: scatter into emergence schedule ---
                for (tt, colofs, gofs, ncol) in em_pieces((t + 1) * Bc, D * Bc):
                    dlo, dhi = gofs // Bc, (gofs + ncol) // Bc
                    nc.tensor.matmul(tt[:, colofs:colofs + ncol],
                                     xbm[:, t * I:(t + 1) * I], G3[:, dlo:dhi, :],
                                     start=False, stop=False,
                                     skip_group_check=True)

                for m in range(4):
                    for k in range(2):
                        for j in range(4):
                            nc.tensor.matmul(
                                p_grz[32 * j:32 * (j + 1), m, :],
                                wcomb[:, k, m * 128 + 32 * j:m * 128 + 32 * (j + 1)],
                                uprime[:, k, :], start=False,
                                stop=(m == 3 and k == 1 and j == 3),
                                skip_group_check=True, tile_position=(0, 32 * j))
                for cch in range(2):
                    for k in range(2):
                        for j in range(4):
                            nc.tensor.matmul(
                                p_gn[32 * j:32 * (j + 1), cch, :],
                                wcomb[:, k, (4 + cch) * 128 + 32 * j:(4 + cch) * 128 + 32 * (j + 1)],
                                uprime[:, k, :], start=False,
                                stop=(cch == 1 and k == 1 and j == 3),
                                skip_group_check=True, tile_position=(0, 32 * j))

                # --- GRU elementwise (sigmoid via tanh: all ACT funcs
                # stay in the exp_and_others table set -> no table reloads) ---
                # th_r = tanh((rpre + b_r)/2)  => r = (th_r+1)/2
                # th_z = tanh((zpre + b_z)/2)  => (1-z) = (1-th_z)/2
                th_r = work.tile([128, 2, Bc], f32, tag="thr")
                th_z = work.tile([128, 2, Bc], f32, tag="thz")
                for cch in range(2):
                    nc.scalar.activation(th_r[:, cch, :], p_grz[:, cch, :],
                                         AF.Tanh,
                                         bias=bsig[:, cch:cch + 1], scale=0.5)
                    nc.scalar.activation(th_z[:, cch, :], p_grz[:, 2 + cch, :],
                                         AF.Tanh,
                                         bias=bsig[:, 2 + cch:3 + cch], scale=0.5)
                hn_b = work.tile([128, 2, Bc], f32, tag="hnb")
                A_ = work.tile([128, 2, Bc], f32, tag="A")
                npre = work.tile([128, 2, Bc], f32, tag="npre")
                n_t = work.tile([128, 2, Bc], f32, tag="n")
                for cch in range(2):
                    # hn_b = h_n + b_hn ; A = (th_r+1)*hn_b = 2*r*hn_b
                    nc.vector.tensor_scalar(hn_b[:, cch, :],
                                            p_gn[:, 2 + cch, :],
                                            bsig[:, 4 + cch:5 + cch], None,
                                            op0=OP.add)
                    nc.gpsimd.tensor_scalar_add(A_[:, cch, :], th_r[:, cch, :],
                                                1.0)
                    nc.gpsimd.tensor_mul(A_[:, cch, :], A_[:, cch, :],
                                         hn_b[:, cch, :])
                    # npre = 2*i_n_raw + A ; n = tanh(npre/2 + b_in)
                    nc.vector.scalar_tensor_tensor(
                        npre[:, cch, :], p_gn[:, cch, :], 2.0,
                        A_[:, cch, :], op0=OP.mult, op1=OP.add)
                    nc.scalar.activation(n_t[:, cch, :], npre[:, cch, :], AF.Tanh,
                                         bias=bsig[:, 6 + cch:7 + cch], scale=0.5)
                # h += m*(1-z)*(n-h) = -0.5 * [ (th_z-1)*m ] * (n-h)
                q = work.tile([128, 2, Bc], f32, tag="q")
                nc.gpsimd.tensor_sub(q[:], n_t[:], h[:])
                w1 = work.tile([128, 2, Bc], f32, tag="w1")
                nc.gpsimd.tensor_scalar(w1[:], th_z[:], 1.0, -0.5,
                                        op0=OP.subtract, op1=OP.mult)
                nc.gpsimd.tensor_mul(w1[:], w1[:], maskr[:, t, :, :])
                uu = work.tile([128, 2, Bc], f32, tag="uu")
                nc.gpsimd.tensor_mul(uu[:], q[:], w1[:])
                nc.gpsimd.tensor_add(h[:], h[:], uu[:])

        # ---------------- decoder prep ----------------
        nc.gpsimd.memset(dec_mem[:], 0.0)
        # decoder mem k: arrival T+1+2k -> emerge col (T+1+2k)*Bc + b
        src0 = (T + 1) * Bc
        if src0 >= emA_cols:
            src = emB[:, src0 - emA_cols:src0 - emA_cols + 2 * Bc * N_NZ]
        else:
            src = emA[:, src0:src0 + 2 * Bc * N_NZ]
        srcv = src.rearrange("p (k x) -> p k x", k=N_NZ)[:, :, 0:Bc]
        nc.vector.tensor_copy(dec_mem[:, 0:N_NZ, :], srcv)

        with tc.tile_pool(name="pdec1", bufs=1, space="PSUM") as pdec1, \
             tc.tile_pool(name="pdec2", bufs=2, space="PSUM") as pdec2:

            p_d1 = pdec1.tile([128, 2, OUT * Bc], f32, tag="d1")
            for m in range(2):
                nc.tensor.matmul(p_d1[:, m, :], we1m[:, m, :],
                                 dec_mem[:].rearrange("p a b -> p (a b)"),
                                 start=True, stop=True)
            e1d = state.tile([128, 2, OUT * Bc], f32, tag="e1d")
            for m in range(2):
                nc.vector.tensor_scalar(e1d[:, m, :], p_d1[:, m, :],
                                        bdec[:, m:m + 1], None, op0=OP.add)
            m3 = state.tile([128, 2, OUT * Bc], f32, tag="m3")
            nc.gpsimd.tensor_scalar_min(m3[:], e1d[:], 0.0)
            e3a = state.tile([128, 2, OUT * Bc], f32, tag="e3a")
            nc.scalar.activation(e3a[:], m3[:], AF.Exp)
            r3a = state.tile([128, 2, OUT * Bc], f32, tag="r3a")
            nc.scalar.activation(r3a[:], e1d[:], AF.Relu)
            udec = state.tile([128, 2, OUT * Bc], f32, tag="udec")
            nc.gpsimd.tensor_add(udec[:], e3a[:], r3a[:])

            for m6 in range(6):
                p_gd = pdec1.tile([128, OUT * Bc], f32, tag="gd")
                for k in range(2):
                    nc.tensor.matmul(p_gd[:], wcomb[:, k, m6 * 128:(m6 + 1) * 128],
                                     udec[:, k, :], start=(k == 0), stop=(k == 1))
                nc.vector.tensor_scalar(
                    gidec[:, m6, :, :].rearrange("p a b -> p (a b)"), p_gd[:],
                    bcombp[:, m6:m6 + 1], None, op0=OP.add)

            # ---------------- decoder loop ----------------
            for k in range(OUT):
                hprev = h[:, :, :] if k == 0 else nh_all[:, :, k - 1, :]
                p_g2 = pdec2.tile([128, 6, Bc], f32, tag="g2")
                nc.vector.memset(p_g2[:], 0.0)
                for m in range(6):
                    for kk in range(2):
                        hk = h[:, kk, :] if k == 0 else nh_all[:, kk, k - 1, :]
                        for j in range(4):
                            nc.tensor.matmul(
                                p_g2[32 * j:32 * (j + 1), m, :],
                                whh[:, kk, m * 128 + 32 * j:m * 128 + 32 * (j + 1)],
                                hk, start=False,
                                stop=(m == 5 and kk == 1 and j == 3),
                                skip_group_check=True, tile_position=(0, 32 * j))
                rzs = work.tile([128, 4, Bc], f32, tag="rzs")
                for j in range(4):
                    nc.vector.tensor_add(rzs[:, j, :], p_g2[:, j, :],
                                         gidec[:, j, k, :])
                th_rd = work.tile([128, 2, Bc], f32, tag="thrd")
                th_zd = work.tile([128, 2, Bc], f32, tag="thzd")
                for cch in range(2):
                    nc.scalar.activation(th_rd[:, cch, :], rzs[:, cch, :],
                                         AF.Tanh, bias=bhhd[:, cch:cch + 1],
                                         scale=0.5)
                    nc.scalar.activation(th_zd[:, cch, :], rzs[:, 2 + cch, :],
                                         AF.Tanh, bias=bhhd[:, 2 + cch:3 + cch],
                                         scale=0.5)
                hn_bd = work.tile([128, 2, Bc], f32, tag="hnbd")
                A_d = work.tile([128, 2, Bc], f32, tag="Ad")
                npred = work.tile([128, 2, Bc], f32, tag="npred")
                ndec = work.tile([128, 2, Bc], f32, tag="ndec")
                for cch in range(2):
                    nc.vector.tensor_scalar(hn_bd[:, cch, :], p_g2[:, 4 + cch, :],
                                            bhhd[:, 4 + cch:5 + cch], None,
                                            op0=OP.add)
                    nc.gpsimd.tensor_scalar_add(A_d[:, cch, :],
                                                th_rd[:, cch, :], 1.0)
                    nc.gpsimd.tensor_mul(A_d[:, cch, :], A_d[:, cch, :],
                                         hn_bd[:, cch, :])
                    # gidec_in already includes b_comb'_n
                    nc.vector.scalar_tensor_tensor(
                        npred[:, cch, :], gidec[:, 4 + cch, k, :], 2.0,
                        A_d[:, cch, :], op0=OP.mult, op1=OP.add)
                nc.scalar.activation(ndec[:], npred[:], AF.Tanh, scale=0.5)
                qd = work.tile([128, 2, Bc], f32, tag="qd")
                nc.gpsimd.tensor_sub(qd[:], ndec[:], hprev)
                w1d = work.tile([128, 2, Bc], f32, tag="w1d")
                nc.gpsimd.tensor_scalar(w1d[:], th_zd[:], 1.0, -0.5,
                                        op0=OP.subtract, op1=OP.mult)
                ud = work.tile([128, 2, Bc], f32, tag="ud")
                nc.gpsimd.tensor_mul(ud[:], qd[:], w1d[:])
                nc.gpsimd.tensor_add(nh_all[:, :, k, :], hprev, ud[:])

            # ---------------- output MLP (batched) ----------------
            p_f1 = pdec1.tile([128, 2, OUT * Bc], f32, tag="f1")
            for mf in range(2):
                for kc in range(2):
                    nc.tensor.matmul(p_f1[:, mf, :],
                                     wf1[:, kc, mf * 128:(mf + 1) * 128],
                                     nh_all[:, kc, :, :].rearrange("p a b -> p (a b)"),
                                     start=(kc == 0), stop=(kc == 1))
            frelu = state.tile([128, 2, OUT * Bc], f32, tag="frelu")
            for mf in range(2):
                nc.scalar.activation(frelu[:, mf, :], p_f1[:, mf, :], AF.Relu,
                                     bias=bf1[:, mf:mf + 1])
            p_f2 = pdec1.tile([C, OUT * Bc], f32, tag="f2")
            for kc in range(2):
                nc.tensor.matmul(p_f2[:], wf2[:, kc, :], frelu[:, kc, :],
                                 start=(kc == 0), stop=(kc == 1))
            outsb = state.tile([C, OUT * Bc], f32, tag="outsb")
            nc.vector.tensor_scalar(outsb[:], p_f2[:], bf2[:], None, op0=OP.add)
            nc.sync.dma_start(d_out[:], outsb[:])

    nc.finalize()
    return nc


def _get_program(T, OUT):
    key = (T, OUT)
    if key not in _PROGRAMS:
        _PROGRAMS[key] = _build_program(T, OUT)
    return _PROGRAMS[key]


def _precompute(inputs):
    x = np.asarray(inputs["x"], dtype=np.float32)
    lengths = np.asarray(inputs["lengths"]).astype(np.int64)
    T = x.shape[1]
    OUT = int(np.asarray(inputs["out_lengths"]))

    W_dg1 = np.asarray(inputs["W_dg1"], np.float32)
    b_dg1 = np.asarray(inputs["b_dg1"], np.float32)
    W_dg2 = np.asarray(inputs["W_dg2"], np.float32)
    b_dg2 = np.asarray(inputs["b_dg2"], np.float32)
    W_e1 = np.asarray(inputs["W_e1"], np.float32)
    b_e1 = np.asarray(inputs["b_e1"], np.float32)
    W_e2 = np.asarray(inputs["W_e2"], np.float32)
    b_e2 = np.asarray(inputs["b_e2"], np.float32)
    W_ih = np.asarray(inputs["W_ih"], np.float32)
    b_ih = np.asarray(inputs["b_ih"], np.float32)
    W_hh = np.asarray(inputs["W_hh"], np.float32)
    b_hh = np.asarray(inputs["b_hh"], np.float32)
    W_f1 = np.asarray(inputs["W_f1"], np.float32)
    b_f1 = np.asarray(inputs["b_f1"], np.float32)
    W_f2 = np.asarray(inputs["W_f2"], np.float32)
    b_f2 = np.asarray(inputs["b_f2"], np.float32)

    import jax
    import jax.numpy as jnp
    cpu = jax.devices("cpu")[0]
    with jax.default_device(cpu):
        gkey = jax.random.key(42)
        g_enc = np.asarray(jax.random.gumbel(
            jax.random.fold_in(gkey, 0), (T, B, D), jnp.float32))

    f64 = np.float64
    decay = (DECAY ** np.arange(1, D + 1, dtype=f64)).astype(np.float32)
    masks = (np.arange(T)[None, :] < lengths[:, None]).astype(np.float32)

    W_dg1x, W_dg1h = W_dg1[:I], W_dg1[I:]
    W_e1x, W_e1m = W_e1[:I], W_e1[I:]

    W_comb64 = W_e2.astype(f64) @ W_ih.astype(f64)
    W_comb = W_comb64.astype(np.float32)
    b_comb64 = b_e2.astype(f64) @ W_ih.astype(f64) + b_ih.astype(f64)
    b_combp = (b_comb64 - W_comb.astype(f64).sum(0)).astype(np.float32)
    b_dg2p = (b_dg2.astype(f64) - W_dg2.astype(f64).sum(0)).astype(np.float32)

    # x-parts, batched over all t
    pre_dg_full = x @ W_dg1x + b_dg1          # [B, T, D]
    pre_e_full = x @ W_e1x + b_e1             # [B, T, H]
    gp_full = g_enc + b_dg2p[None, None, :]   # [T, B, D]

    b_hh_r, b_hh_z, b_hh_n = b_hh[:H], b_hh[H:2 * H], b_hh[2 * H:]
    b_cp_r, b_cp_z, b_cp_n = b_combp[:H], b_combp[H:2 * H], b_combp[2 * H:]

    def chunks2(vec):  # [256] -> [128, 2]
        return np.ascontiguousarray(vec.reshape(2, 128).T.astype(np.float32))

    bsig = np.concatenate([
        chunks2(0.5 * (b_cp_r + b_hh_r)), chunks2(0.5 * (b_cp_z + b_hh_z)),
        chunks2(b_hh_n), chunks2(b_cp_n)], axis=1)      # [128, 8]
    bhhd = np.concatenate([
        chunks2(0.5 * b_hh_r), chunks2(0.5 * b_hh_z),
        chunks2(b_hh_n)], axis=1)  # [128, 6]
    bdec = chunks2(b_e1)
    bcombp_d = np.ascontiguousarray(b_combp.reshape(6, 128).T)
    bf1_d = chunks2(b_f1)
    bf2_d = b_f2.reshape(C, 1).copy()

    eyedecay = np.zeros((Bc, D, Bc), np.float32)
    for b in range(Bc):
        eyedecay[b, :, b] = decay
    shared = dict(
        eyedecay=eyedecay,
        wdg1h=W_dg1h.reshape(2, 128, D).transpose(1, 0, 2).copy(),
        wdg2=W_dg2.copy(),
        we1m=W_e1m.reshape(I, 2, 128).copy(),
        wcomb=W_comb.reshape(2, 128, 3 * H).transpose(1, 0, 2).copy(),
        whh=W_hh.reshape(2, 128, 3 * H).transpose(1, 0, 2).copy(),
        wf1=W_f1.reshape(2, 128, H).transpose(1, 0, 2).copy(),
        wf2=W_f2.reshape(2, 128, C).transpose(1, 0, 2).copy(),
        bsig=bsig, bdec=bdec, bhhd=bhhd, bcombp=bcombp_d,
        bf1=bf1_d, bf2=bf2_d,
        zrow=np.zeros((1, I), np.float32),
        zcols=np.zeros((1, 512), np.float32),
    )

    in_maps = []
    for cc in range(NCORE):
        bsl = slice(cc * Bc, (cc + 1) * Bc)
        m = dict(shared)
        m["pre_dg"] = np.ascontiguousarray(
            pre_dg_full[bsl].transpose(2, 1, 0).reshape(D, T * Bc))
        m["pre_e"] = np.ascontiguousarray(
            pre_e_full[bsl].transpose(2, 1, 0)       # [H, T, Bc]
            .reshape(2, 128, T, Bc).transpose(1, 0, 2, 3))
        m["gp"] = np.ascontiguousarray(
            gp_full[:, bsl, :].transpose(1, 0, 2).reshape(Bc, T * D))
        m["maskr"] = np.ascontiguousarray(np.broadcast_to(
            masks[bsl].T[None, :, None, :], (128, T, 2, Bc)))
        m["xbm"] = np.ascontiguousarray(x[bsl].reshape(Bc, T * I))
        in_maps.append(m)
    return in_maps, T, OUT


def kernel(**inputs):
    global LAST_RESULTS
    from concourse.bass_utils import run_bass_kernel_spmd

    in_maps, T, OUT = _precompute(inputs)
    nc = _get_program(T, OUT)
    res = run_bass_kernel_spmd(nc, in_maps, core_ids=list(range(NCORE)))
    LAST_RESULTS = res

    out_full = np.zeros((B, OUT, C), np.float32)
    idx = np.arange(OUT) * Bc
    for cc in range(NCORE):
        oc = res.results[cc]["out"]          # [C, OUT*Bc]
        for b in range(Bc):
            out_full[cc * Bc + b] = oc[:, idx + b].T
    return out_full


# revision 27
# speedup vs baseline: 1.0099x; 1.0099x over previous
"""DelayRNN Trainium2 kernel.

Algorithm notes (derived from the reference semantics):
  - The [B, D+1, D, I] buffer is a write-once delay line: at encoder step t the
    (one-hot) gate picks delay d*, and x_t * 0.99^(d*+1) is read back into
    `mem` exactly at step t + d* + 1.  We therefore keep only an "emergence
    schedule" E[arrival, b, :I] and accumulate outer products into it.
  - Decoder steps shift twice and read after the first shift, so decoder step k
    reads arrival T + 1 + 2k (zero for k >= 16).  Decoder writes are x_zero ->
    the whole decoder gate path is dead code.
  - gumbel noise is a constant (jax key 42); computed on host.
  - rnn_in only feeds the GRU, so W_e2 @ W_ih is fused host-side (W_comb).
  - elu(x) = relu(x) + exp(min(x, 0)) - 1; the -1 is folded into downstream
    biases (colsum of the following weight matrix).

Layout: feature-major [feature-partitions, batch-free], 8-way batch-parallel
(4 rows/core).  The emergence schedule lives in PSUM and is accumulated
directly by the scatter matmuls (start=False).
"""

import sys

import numpy as np

try:
    import concourse.bass  # noqa: F401
except ImportError:
    for p in ("/opt/trn_rl_repo", "/root/.axon_site/_ro/trn_rl_repo"):
        if p not in sys.path:
            sys.path.insert(0, p)

B, T0, I, H, D, C, OUT0 = 32, 128, 64, 256, 32, 64, 32
NCORE = 8
Bc = B // NCORE
DECAY = 0.99

_PROGRAMS = {}
LAST_RESULTS = None


def _build_program(T, OUT):
    import concourse.bass as bass
    import concourse.bacc as bacc
    import concourse.tile as tile
    from concourse import mybir
    from contextlib import ExitStack

    f32 = mybir.dt.float32
    AX = mybir.AxisListType
    OP = mybir.AluOpType
    AF = mybir.ActivationFunctionType

    EMTOT = (T + D + 1) * Bc          # emergence schedule columns (arrival*Bc+b)
    emA_cols = min(EMTOT, 512)
    emB_cols = EMTOT - emA_cols
    N_NZ = min(OUT, (D - 2) // 2 + 1)  # decoder steps with nonzero mem (16)

    nc = bacc.Bacc()

    dI = {}
    def din(name, shape):
        dI[name] = nc.dram_tensor(name, list(shape), f32, kind="ExternalInput")
        return dI[name]

    d_pre_dg = din("pre_dg", [D, T * Bc])
    d_pre_e = din("pre_e", [128, 2, T, Bc])
    d_gp = din("gp", [Bc, T * D])
    d_maskr = din("maskr", [128, T, 2, Bc])
    d_xbm = din("xbm", [Bc, T * I])
    d_eyedecay = din("eyedecay", [Bc, D, Bc])
    d_wdg1h = din("wdg1h", [128, 2, D])
    d_wdg2 = din("wdg2", [D, D])
    d_we1m = din("we1m", [I, 2, 128])
    d_wcomb = din("wcomb", [128, 2, 3 * H])
    d_whh = din("whh", [128, 2, 3 * H])
    d_wf1 = din("wf1", [128, 2, H])
    d_wf2 = din("wf2", [128, 2, C])
    d_bsig = din("bsig", [128, 8])
    d_bdec = din("bdec", [128, 2])
    d_bhhd = din("bhhd", [128, 6])
    d_bcombp = din("bcombp", [128, 6])
    d_bf1 = din("bf1", [128, 2])
    d_bf2 = din("bf2", [C, 1])
    d_zrow = din("zrow", [1, I])
    d_zcols = din("zcols", [1, 512])
    d_out = nc.dram_tensor("out", [C, OUT * Bc], f32, kind="ExternalOutput")

    with ExitStack() as ctx:
        tc = ctx.enter_context(tile.TileContext(nc))

        emerge_pool = ctx.enter_context(
            tc.tile_pool(name="emerge", bufs=1, space="PSUM"))
        emA = emerge_pool.tile([I, emA_cols], f32, tag="emA")
        if emB_cols:
            emB = emerge_pool.tile([I, emB_cols], f32, tag="emB")
        else:
            emB = None

        consts = ctx.enter_context(tc.tile_pool(name="consts", bufs=1))
        state = ctx.enter_context(tc.tile_pool(name="state", bufs=1))
        work = ctx.enter_context(tc.tile_pool(name="work", bufs=3))

        def load(dram, shape):
            t = consts.tile(list(shape), f32, tag=dram.name + "_c")
            nc.sync.dma_start(t[:], dram[:])
            return t

        pre_dg = load(d_pre_dg, [D, T * Bc])
        pre_e = load(d_pre_e, [128, 2, T, Bc])
        gp = load(d_gp, [Bc, T * D])
        maskr = load(d_maskr, [128, T, 2, Bc])
        xbm = load(d_xbm, [Bc, T * I])
        eyedecay = load(d_eyedecay, [Bc, D, Bc])
        wdg1h = load(d_wdg1h, [128, 2, D])
        wdg2 = load(d_wdg2, [D, D])
        we1m = load(d_we1m, [I, 2, 128])
        wcomb = load(d_wcomb, [128, 2, 3 * H])
        whh = load(d_whh, [128, 2, 3 * H])
        wf1 = load(d_wf1, [128, 2, H])
        wf2 = load(d_wf2, [128, 2, C])
        bsig = load(d_bsig, [128, 8])
        bdec = load(d_bdec, [128, 2])
        bhhd = load(d_bhhd, [128, 6])
        bcombp = load(d_bcombp, [128, 6])
        bf1 = load(d_bf1, [128, 2])
        bf2 = load(d_bf2, [C, 1])
        zrow = load(d_zrow, [1, I])
        zcols = load(d_zcols, [1, 512])

        h = state.tile([128, 2, Bc], f32, tag="h")
        zconst = state.tile([128, 4, Bc], f32, tag="zconst")
        nc.gpsimd.memset(zconst[:], 0.0)

        nh_all = state.tile([128, 2, OUT, Bc], f32, tag="nh_all")
        dec_mem = state.tile([I, OUT, Bc], f32, tag="dec_mem")
        gidec = state.tile([128, 6, OUT, Bc], f32, tag="gidec")

        nc.gpsimd.memset(h[:], 0.0)

        # zero the emergence schedule (sets has_written for accumulation)
        nc.tensor.matmul(emA[:, 0:emA_cols], zrow[:], zcols[:, 0:emA_cols],
                         start=True, stop=True, skip_group_check=True)
        if emB is not None:
            nc.tensor.matmul(emB[:, 0:emB_cols], zrow[:], zcols[:, 0:emB_cols],
                             start=True, stop=True, skip_group_check=True)

        def em_pieces(col0, ncols):
            """split [col0, col0+ncols) emerge columns into per-tensor pieces"""
            out = []
            if col0 < emA_cols:
                n1 = min(ncols, emA_cols - col0)
                out.append((emA, col0, 0, n1))
                if n1 < ncols:
                    out.append((emB, 0, n1, ncols - n1))
            else:
                out.append((emB, col0 - emA_cols, 0, ncols))
            return out

        # ---------------- encoder ----------------
        with tc.tile_pool(name="penc", bufs=2, space="PSUM") as penc, \
             tc.tile_pool(name="penc1", bufs=1, space="PSUM") as penc1:
            for t in range(T):
                pgate = penc.tile([D, 4 + D], f32, tag="gate")
                p_l1 = pgate[:, 0:Bc]
                p_l2 = pgate[0:Bc, Bc:Bc + D]
                p_e1 = penc1.tile([128, 2, Bc], f32, tag="e1")
                p_grz = penc.tile([128, 4, Bc], f32, tag="grz")
                p_gn = penc1.tile([128, 4, Bc], f32, tag="gn")

                # zero the bank memory on DVE; all matmuls below use
                # start=False (overwrite-on-stale-has_written, then accumulate)
                nc.scalar.activation(p_grz[:], zconst[:], AF.Identity)
                nc.scalar.activation(p_gn[:], zconst[:], AF.Identity)

                # --- PE: l1h first: the gate chain (elu->l2->argmax->G3)
                # runs on ACT/DVE/Pool underneath gh/gi ---
                for k in range(2):
                    nc.tensor.matmul(p_l1, wdg1h[:, k, :], h[:, k, :],
                                     start=(k == 0), stop=(k == 1))

                # --- PE: gh (depends only on h); 4x col-tiling so the
                # stationary loads stream via 4 XBUSes concurrently ---
                for m in range(4):
                    for k in range(2):
                        for j in range(4):
                            nc.tensor.matmul(
                                p_grz[32 * j:32 * (j + 1), m, :],
                                whh[:, k, m * 128 + 32 * j:m * 128 + 32 * (j + 1)],
                                h[:, k, :], start=False, stop=False,
                                skip_group_check=True, tile_position=(0, 32 * j))
                for cch in range(2):
                    for k in range(2):
                        for j in range(4):
                            nc.tensor.matmul(
                                p_gn[32 * j:32 * (j + 1), 2 + cch, :],
                                whh[:, k, (4 + cch) * 128 + 32 * j:(4 + cch) * 128 + 32 * (j + 1)],
                                h[:, k, :], start=False, stop=False,
                                skip_group_check=True, tile_position=(0, 32 * j))
                # mem for this step (written by scatter of step t-1)
                mem_sb = work.tile([I, Bc], f32, tag="mem")
                nc.vector.tensor_copy(mem_sb[:], emA[:, t * Bc:(t + 1) * Bc])

                # --- PE: e1m (col-tiled) ---
                for m in range(2):
                    for j in range(4):
                        nc.tensor.matmul(
                            p_e1[32 * j:32 * (j + 1), m, :],
                            we1m[:, m, 32 * j:32 * (j + 1)], mem_sb[:],
                            start=True, stop=True, skip_group_check=True,
                            tile_position=(0, 32 * j))

                # --- gate path elementwise ---
                l1s = work.tile([D, Bc], f32, tag="l1s")
                nc.vector.tensor_add(l1s[:], p_l1, pre_dg[:, t * Bc:(t + 1) * Bc])
                m1 = work.tile([D, Bc], f32, tag="m1")
                nc.gpsimd.tensor_scalar_min(m1[:], l1s[:], 0.0)
                e1a = work.tile([D, Bc], f32, tag="e1a")
                nc.scalar.activation(e1a[:], m1[:], AF.Exp)
                r1a = work.tile([D, Bc], f32, tag="r1a")
                nc.scalar.activation(r1a[:], l1s[:], AF.Relu)
                celu = work.tile([D, Bc], f32, tag="celu")
                nc.gpsimd.tensor_add(celu[:], e1a[:], r1a[:])

                # --- PE: l2 (logits, batch-major out) ---
                nc.tensor.matmul(p_l2, celu[:], wdg2[:], start=True, stop=True)

                v = work.tile([Bc, D], f32, tag="v")
                nc.vector.tensor_add(v[:], p_l2, gp[:, t * D:(t + 1) * D])
                mx = work.tile([Bc, 1], f32, tag="mx")
                nc.vector.tensor_reduce(mx[:], v[:], axis=AX.X, op=OP.max)
                oh = work.tile([Bc, D], f32, tag="oh")
                nc.vector.tensor_scalar(oh[:], v[:], mx[:], None, op0=OP.is_equal)
                G3 = work.tile([Bc, D, Bc], f32, tag="G3")
                nc.vector.tensor_mul(G3[:], oh[:].broadcast_to((Bc, D, Bc)),
                                     eyedecay[:])

                # --- e path elementwise ---
                e1s = work.tile([128, 2, Bc], f32, tag="e1s")
                nc.vector.tensor_add(e1s[:], p_e1[:], pre_e[:, :, t, :])
                m2 = work.tile([128, 2, Bc], f32, tag="m2")
                nc.gpsimd.tensor_scalar_min(m2[:], e1s[:], 0.0)
                e2a = work.tile([128, 2, Bc], f32, tag="e2a")
                nc.scalar.activation(e2a[:], m2[:], AF.Exp)
                r2a = work.tile([128, 2, Bc], f32, tag="r2a")
                nc.scalar.activation(r2a[:], e1s[:], AF.Relu)
                uprime = work.tile([128, 2, Bc], f32, tag="uprime")
                nc.gpsimd.tensor_add(uprime[:], e2a[:], r2a[:])

                # --- PE: gi ---
                # --- PE: scatter into emergence schedule ---
                for (tt, colofs, gofs, ncol) in em_pieces((t + 1) * Bc, D * Bc):
                    dlo, dhi = gofs // Bc, (gofs + ncol) // Bc
                    nc.tensor.matmul(tt[:, colofs:colofs + ncol],
                                     xbm[:, t * I:(t + 1) * I], G3[:, dlo:dhi, :],
                                     start=False, stop=False,
                                     skip_group_check=True)

                for m in range(4):
                    for k in range(2):
                        for j in range(4):
                            nc.tensor.matmul(
                                p_grz[32 * j:32 * (j + 1), m, :],
                                wcomb[:, k, m * 128 + 32 * j:m * 128 + 32 * (j + 1)],
                                uprime[:, k, :], start=False,
                                stop=(m == 3 and k == 1 and j == 3),
                                skip_group_check=True, tile_position=(0, 32 * j))
                for cch in range(2):
                    for k in range(2):
                        for j in range(4):
                            nc.tensor.matmul(
                                p_gn[32 * j:32 * (j + 1), cch, :],
                                wcomb[:, k, (4 + cch) * 128 + 32 * j:(4 + cch) * 128 + 32 * (j + 1)],
                                uprime[:, k, :], start=False,
                                stop=(cch == 1 and k == 1 and j == 3),
                                skip_group_check=True, tile_position=(0, 32 * j))

                # --- GRU elementwise (sigmoid via tanh: all ACT funcs
                # stay in the exp_and_others table set -> no table reloads) ---
                # th_r = tanh((rpre + b_r)/2)  => r = (th_r+1)/2
                # th_z = tanh((zpre + b_z)/2)  => (1-z) = (1-th_z)/2
                th_r = work.tile([128, 2, Bc], f32, tag="thr")
                th_z = work.tile([128, 2, Bc], f32, tag="thz")
                for cch in range(2):
                    nc.scalar.activation(th_r[:, cch, :], p_grz[:, cch, :],
                                         AF.Tanh,
                                         bias=bsig[:, cch:cch + 1], scale=0.5)
                    nc.scalar.activation(th_z[:, cch, :], p_grz[:, 2 + cch, :],
                                         AF.Tanh,
                                         bias=bsig[:, 2 + cch:3 + cch], scale=0.5)
                hn_b = work.tile([128, 2, Bc], f32, tag="hnb")
                A_ = work.tile([128, 2, Bc], f32, tag="A")
                npre = work.tile([128, 2, Bc], f32, tag="npre")
                n_t = work.tile([128, 2, Bc], f32, tag="n")
                for cch in range(2):
                    # hn_b = h_n + b_hn ; A = (th_r+1)*hn_b = 2*r*hn_b
                    nc.vector.tensor_scalar(hn_b[:, cch, :],
                                            p_gn[:, 2 + cch, :],
                                            bsig[:, 4 + cch:5 + cch], None,
                                            op0=OP.add)
                    nc.gpsimd.tensor_scalar_add(A_[:, cch, :], th_r[:, cch, :],
                                                1.0)
                    nc.gpsimd.tensor_mul(A_[:, cch, :], A_[:, cch, :],
                                         hn_b[:, cch, :])
                    # npre = 2*i_n_raw + A ; n = tanh(npre/2 + b_in)
                    nc.vector.scalar_tensor_tensor(
                        npre[:, cch, :], p_gn[:, cch, :], 2.0,
                        A_[:, cch, :], op0=OP.mult, op1=OP.add)
                    nc.scalar.activation(n_t[:, cch, :], npre[:, cch, :], AF.Tanh,
                                         bias=bsig[:, 6 + cch:7 + cch], scale=0.5)
                # h += m*(1-z)*(n-h) = -0.5 * [ (th_z-1)*m ] * (n-h)
                q = work.tile([128, 2, Bc], f32, tag="q")
                nc.gpsimd.tensor_sub(q[:], n_t[:], h[:])
                w1 = work.tile([128, 2, Bc], f32, tag="w1")
                nc.gpsimd.tensor_scalar(w1[:], th_z[:], 1.0, -0.5,
                                        op0=OP.subtract, op1=OP.mult)
                nc.gpsimd.tensor_mul(w1[:], w1[:], maskr[:, t, :, :])
                uu = work.tile([128, 2, Bc], f32, tag="uu")
                nc.gpsimd.tensor_mul(uu[:], q[:], w1[:])
                nc.gpsimd.tensor_add(h[:], h[:], uu[:])

        # ---------------- decoder prep ----------------
        nc.gpsimd.memset(dec_mem[:], 0.0)
        # decoder mem k: arrival T+1+2k -> emerge col (T+1+2k)*Bc + b
        src0 = (T + 1) * Bc
        if src0 >= emA_cols:
            src = emB[:, src0 - emA_cols:src0 - emA_cols + 2 * Bc * N_NZ]
        else:
            src = emA[:, src0:src0 + 2 * Bc * N_NZ]
        srcv = src.rearrange("p (k x) -> p k x", k=N_NZ)[:, :, 0:Bc]
        nc.vector.tensor_copy(dec_mem[:, 0:N_NZ, :], srcv)

        with tc.tile_pool(name="pdec1", bufs=1, space="PSUM") as pdec1, \
             tc.tile_pool(name="pdec2", bufs=2, space="PSUM") as pdec2:

            p_d1 = pdec1.tile([128, 2, OUT * Bc], f32, tag="d1")
            for m in range(2):
                nc.tensor.matmul(p_d1[:, m, :], we1m[:, m, :],
                                 dec_mem[:].rearrange("p a b -> p (a b)"),
                                 start=True, stop=True)
            e1d = state.tile([128, 2, OUT * Bc], f32, tag="e1d")
            for m in range(2):
                nc.vector.tensor_scalar(e1d[:, m, :], p_d1[:, m, :],
                                        bdec[:, m:m + 1], None, op0=OP.add)
            m3 = state.tile([128, 2, OUT * Bc], f32, tag="m3")
            nc.gpsimd.tensor_scalar_min(m3[:], e1d[:], 0.0)
            e3a = state.tile([128, 2, OUT * Bc], f32, tag="e3a")
            nc.scalar.activation(e3a[:], m3[:], AF.Exp)
            r3a = state.tile([128, 2, OUT * Bc], f32, tag="r3a")
            nc.scalar.activation(r3a[:], e1d[:], AF.Relu)
            udec = state.tile([128, 2, OUT * Bc], f32, tag="udec")
            nc.gpsimd.tensor_add(udec[:], e3a[:], r3a[:])

            for m6 in range(6):
                p_gd = pdec1.tile([128, OUT * Bc], f32, tag="gd")
                for k in range(2):
                    nc.tensor.matmul(p_gd[:], wcomb[:, k, m6 * 128:(m6 + 1) * 128],
                                     udec[:, k, :], start=(k == 0), stop=(k == 1))
                nc.vector.tensor_scalar(
                    gidec[:, m6, :, :].rearrange("p a b -> p (a b)"), p_gd[:],
                    bcombp[:, m6:m6 + 1], None, op0=OP.add)

            # ---------------- decoder loop ----------------
            for k in range(OUT):
                hprev = h[:, :, :] if k == 0 else nh_all[:, :, k - 1, :]
                p_g2 = pdec2.tile([128, 6, Bc], f32, tag="g2")
                nc.vector.memset(p_g2[:], 0.0)
                for m in range(6):
                    for kk in range(2):
                        hk = h[:, kk, :] if k == 0 else nh_all[:, kk, k - 1, :]
                        for j in range(4):
                            nc.tensor.matmul(
                                p_g2[32 * j:32 * (j + 1), m, :],
                                whh[:, kk, m * 128 + 32 * j:m * 128 + 32 * (j + 1)],
                                hk, start=False,
                                stop=(m == 5 and kk == 1 and j == 3),
                                skip_group_check=True, tile_position=(0, 32 * j))
                rzs = work.tile([128, 4, Bc], f32, tag="rzs")
                for j in range(4):
                    nc.vector.tensor_add(rzs[:, j, :], p_g2[:, j, :],
                                         gidec[:, j, k, :])
                th_rd = work.tile([128, 2, Bc], f32, tag="thrd")
                th_zd = work.tile([128, 2, Bc], f32, tag="thzd")
                for cch in range(2):
                    nc.scalar.activation(th_rd[:, cch, :], rzs[:, cch, :],
                                         AF.Tanh, bias=bhhd[:, cch:cch + 1],
                                         scale=0.5)
                    nc.scalar.activation(th_zd[:, cch, :], rzs[:, 2 + cch, :],
                                         AF.Tanh, bias=bhhd[:, 2 + cch:3 + cch],
                                         scale=0.5)
                hn_bd = work.tile([128, 2, Bc], f32, tag="hnbd")
                A_d = work.tile([128, 2, Bc], f32, tag="Ad")
                npred = work.tile([128, 2, Bc], f32, tag="npred")
                ndec = work.tile([128, 2, Bc], f32, tag="ndec")
                for cch in range(2):
                    nc.vector.tensor_scalar(hn_bd[:, cch, :], p_g2[:, 4 + cch, :],
                                            bhhd[:, 4 + cch:5 + cch], None,
                                            op0=OP.add)
                    nc.gpsimd.tensor_scalar_add(A_d[:, cch, :],
                                                th_rd[:, cch, :], 1.0)
                    nc.gpsimd.tensor_mul(A_d[:, cch, :], A_d[:, cch, :],
                                         hn_bd[:, cch, :])
                    # gidec_in already includes b_comb'_n
                    nc.vector.scalar_tensor_tensor(
                        npred[:, cch, :], gidec[:, 4 + cch, k, :], 2.0,
                        A_d[:, cch, :], op0=OP.mult, op1=OP.add)
                nc.scalar.activation(ndec[:], npred[:], AF.Tanh, scale=0.5)
                qd = work.tile([128, 2, Bc], f32, tag="qd")
                nc.gpsimd.tensor_sub(qd[:], ndec[:], hprev)
                w1d = work.tile([128, 2, Bc], f32, tag="w1d")
                nc.gpsimd.tensor_scalar(w1d[:], th_zd[:], 1.0, -0.5,
                                        op0=OP.subtract, op1=OP.mult)
                ud = work.tile([128, 2, Bc], f32, tag="ud")
                nc.gpsimd.tensor_mul(ud[:], qd[:], w1d[:])
                nc.gpsimd.tensor_add(nh_all[:, :, k, :], hprev, ud[:])

            # ---------------- output MLP (batched) ----------------
            p_f1 = pdec1.tile([128, 2, OUT * Bc], f32, tag="f1")
            for mf in range(2):
                for kc in range(2):
                    nc.tensor.matmul(p_f1[:, mf, :],
                                     wf1[:, kc, mf * 128:(mf + 1) * 128],
                                     nh_all[:, kc, :, :].rearrange("p a b -> p (a b)"),
                                     start=(kc == 0), stop=(kc == 1))
            frelu = state.tile([128, 2, OUT * Bc], f32, tag="frelu")
            for mf in range(2):
                nc.scalar.activation(frelu[:, mf, :], p_f1[:, mf, :], AF.Relu,
                                     bias=bf1[:, mf:mf + 1])
            p_f2 = pdec1.tile([C, OUT * Bc], f32, tag="f2")
            for kc in range(2):
                nc.tensor.matmul(p_f2[:], wf2[:, kc, :], frelu[:, kc, :],
                                 start=(kc == 0), stop=(kc == 1))
            outsb = state.tile([C, OUT * Bc], f32, tag="outsb")
            nc.vector.tensor_scalar(outsb[:], p_f2[:], bf2[:], None, op0=OP.add)
            nc.sync.dma_start(d_out[:], outsb[:])

    nc.finalize()
    return nc


def _get_program(T, OUT):
    key = (T, OUT)
    if key not in _PROGRAMS:
        _PROGRAMS[key] = _build_program(T, OUT)
    return _PROGRAMS[key]


def _precompute(inputs):
    x = np.asarray(inputs["x"], dtype=np.float32)
    lengths = np.asarray(inputs["lengths"]).astype(np.int64)
    T = x.shape[1]
    OUT = int(np.asarray(inputs["out_lengths"]))

    W_dg1 = np.asarray(inputs["W_dg1"], np.float32)
    b_dg1 = np.asarray(inputs["b_dg1"], np.float32)
    W_dg2 = np.asarray(inputs["W_dg2"], np.float32)
    b_dg2 = np.asarray(inputs["b_dg2"], np.float32)
    W_e1 = np.asarray(inputs["W_e1"], np.float32)
    b_e1 = np.asarray(inputs["b_e1"], np.float32)
    W_e2 = np.asarray(inputs["W_e2"], np.float32)
    b_e2 = np.asarray(inputs["b_e2"], np.float32)
    W_ih = np.asarray(inputs["W_ih"], np.float32)
    b_ih = np.asarray(inputs["b_ih"], np.float32)
    W_hh = np.asarray(inputs["W_hh"], np.float32)
    b_hh = np.asarray(inputs["b_hh"], np.float32)
    W_f1 = np.asarray(inputs["W_f1"], np.float32)
    b_f1 = np.asarray(inputs["b_f1"], np.float32)
    W_f2 = np.asarray(inputs["W_f2"], np.float32)
    b_f2 = np.asarray(inputs["b_f2"], np.float32)

    import jax
    import jax.numpy as jnp
    cpu = jax.devices("cpu")[0]
    with jax.default_device(cpu):
        gkey = jax.random.key(42)
        g_enc = np.asarray(jax.random.gumbel(
            jax.random.fold_in(gkey, 0), (T, B, D), jnp.float32))

    f64 = np.float64
    decay = (DECAY ** np.arange(1, D + 1, dtype=f64)).astype(np.float32)
    masks = (np.arange(T)[None, :] < lengths[:, None]).astype(np.float32)

    W_dg1x, W_dg1h = W_dg1[:I], W_dg1[I:]
    W_e1x, W_e1m = W_e1[:I], W_e1[I:]

    W_comb64 = W_e2.astype(f64) @ W_ih.astype(f64)
    W_comb = W_comb64.astype(np.float32)
    b_comb64 = b_e2.astype(f64) @ W_ih.astype(f64) + b_ih.astype(f64)
    b_combp = (b_comb64 - W_comb.astype(f64).sum(0)).astype(np.float32)
    b_dg2p = (b_dg2.astype(f64) - W_dg2.astype(f64).sum(0)).astype(np.float32)

    # x-parts, batched over all t
    pre_dg_full = x @ W_dg1x + b_dg1          # [B, T, D]
    pre_e_full = x @ W_e1x + b_e1             # [B, T, H]
    gp_full = g_enc + b_dg2p[None, None, :]   # [T, B, D]

    b_hh_r, b_hh_z, b_hh_n = b_hh[:H], b_hh[H:2 * H], b_hh[2 * H:]
    b_cp_r, b_cp_z, b_cp_n = b_combp[:H], b_combp[H:2 * H], b_combp[2 * H:]

    def chunks2(vec):  # [256] -> [128, 2]
        return np.ascontiguousarray(vec.reshape(2, 128).T.astype(np.float32))

    bsig = np.concatenate([
        chunks2(0.5 * (b_cp_r + b_hh_r)), chunks2(0.5 * (b_cp_z + b_hh_z)),
        chunks2(b_hh_n), chunks2(b_cp_n)], axis=1)      # [128, 8]
    bhhd = np.concatenate([
        chunks2(0.5 * b_hh_r), chunks2(0.5 * b_hh_z),
        chunks2(b_hh_n)], axis=1)  # [128, 6]
    bdec = chunks2(b_e1)
    bcombp_d = np.ascontiguousarray(b_combp.reshape(6, 128).T)
    bf1_d = chunks2(b_f1)
    bf2_d = b_f2.reshape(C, 1).copy()

    eyedecay = np.zeros((Bc, D, Bc), np.float32)
    for b in range(Bc):
        eyedecay[b, :, b] = decay
    shared = dict(
        eyedecay=eyedecay,
        wdg1h=W_dg1h.reshape(2, 128, D).transpose(1, 0, 2).copy(),
        wdg2=W_dg2.copy(),
        we1m=W_e1m.reshape(I, 2, 128).copy(),
        wcomb=W_comb.reshape(2, 128, 3 * H).transpose(1, 0, 2).copy(),
        whh=W_hh.reshape(2, 128, 3 * H).transpose(1, 0, 2).copy(),
        wf1=W_f1.reshape(2, 128, H).transpose(1, 0, 2).copy(),
        wf2=W_f2.reshape(2, 128, C).transpose(1, 0, 2).copy(),
        bsig=bsig, bdec=bdec, bhhd=bhhd, bcombp=bcombp_d,
        bf1=bf1_d, bf2=bf2_d,
        zrow=np.zeros((1, I), np.float32),
        zcols=np.zeros((1, 512), np.float32),
    )

    in_maps = []
    for cc in range(NCORE):
        bsl = slice(cc * Bc, (cc + 1) * Bc)
        m = dict(shared)
        m["pre_dg"] = np.ascontiguousarray(
            pre_dg_full[bsl].transpose(2, 1, 0).reshape(D, T * Bc))
        m["pre_e"] = np.ascontiguousarray(
            pre_e_full[bsl].transpose(2, 1, 0)       # [H, T, Bc]
            .reshape(2, 128, T, Bc).transpose(1, 0, 2, 3))
        m["gp"] = np.ascontiguousarray(
            gp_full[:, bsl, :].transpose(1, 0, 2).reshape(Bc, T * D))
        m["maskr"] = np.ascontiguousarray(np.broadcast_to(
            masks[bsl].T[None, :, None, :], (128, T, 2, Bc)))
        m["xbm"] = np.ascontiguousarray(x[bsl].reshape(Bc, T * I))
        in_maps.append(m)
    return in_maps, T, OUT


def kernel(**inputs):
    global LAST_RESULTS
    from concourse.bass_utils import run_bass_kernel_spmd

    in_maps, T, OUT = _precompute(inputs)
    nc = _get_program(T, OUT)
    res = run_bass_kernel_spmd(nc, in_maps, core_ids=list(range(NCORE)))
    LAST_RESULTS = res

    out_full = np.zeros((B, OUT, C), np.float32)
    idx = np.arange(OUT) * Bc
    for cc in range(NCORE):
        oc = res.results[cc]["out"]          # [C, OUT*Bc]
        for b in range(Bc):
            out_full[cc * Bc + b] = oc[:, idx + b].T
    return out_full
